# revision 16
# baseline (speedup 1.0000x reference)
"""DeepSeekV2 MoE layer on 8 trn2 NeuronCores (expert-parallel).

Strategy (v3):
  - Host: gate softmax + group-limited top-k routing -> per-expert token index
    lists and combine weights (control data only; all heavy FLOPs on device).
  - Device (SPMD over 8 cores, 4 experts each; expert groups == cores):
      zero 4 column-sharded routed-partial tensors y_n [T, 512] (Scalar queue);
      per expert: transposed fp16 dma_gather per token chunk (tokens land
      H-tiled on partitions) -> mm1/mm3 fp16 -> silu*mul -> fp16 mm4 ->
      scale by combine weight -> one batched dma_scatter_add per (e, n);
      shared-expert intermediate (fp32r) for own 512-token slice;
      4x ReduceScatter(add) over cores (routed only) overlap the shared
      output matmuls; out = rs_n + shared.
  - Host: concatenate 512-row slices -> [B, S, H].
"""
import sys

import numpy as np

sys.path.insert(0, "/opt/trn_rl_repo")

import concourse.bass as bass
import concourse.mybir as mybir
import concourse.tile as tile
from concourse import bacc
from concourse.bass_utils import run_bass_kernel_spmd

F32 = mybir.dt.float32
F32R = mybir.dt.float32r
FP16 = mybir.dt.float16
I16 = mybir.dt.int16
AF = mybir.ActivationFunctionType
OP = mybir.AluOpType

N_GROUP, TOPK_GROUP, TOP_K = 8, 3, 6
NCORES = 8


def _routing(x, gate_w):
    T, E = x.shape[0], gate_w.shape[0]
    logits = (x @ gate_w.T).astype(np.float64)
    e = np.exp(logits - logits.max(-1, keepdims=True))
    scores = e / e.sum(-1, keepdims=True)
    per_group = E // N_GROUP
    group_scores = scores.reshape(T, N_GROUP, per_group).max(-1)
    order = np.argsort(-group_scores, axis=-1, kind="stable")
    group_mask = np.zeros((T, N_GROUP), bool)
    np.put_along_axis(group_mask, order[:, :TOPK_GROUP], True, axis=1)
    tmp = np.where(np.repeat(group_mask, per_group, axis=1), scores, 0.0)
    order_e = np.argsort(-tmp, axis=-1, kind="stable")
    topk_idx = order_e[:, :TOP_K]
    topk_w = np.take_along_axis(tmp, topk_idx, axis=1)
    topk_w = topk_w / (topk_w.sum(-1, keepdims=True) + 1e-20)
    combine = np.zeros((T, E), np.float32)
    np.put_along_axis(combine, topk_idx, topk_w.astype(np.float32), axis=1)
    return combine


def _chunks(cap):
    out, rem = [], cap
    while rem:
        if rem <= 512:
            out.append(rem)
            rem = 0
        elif rem == 640:
            out.append(384)
            rem = 256
        else:
            out.append(512)
            rem -= 512
    return out


def build_kernel(T, H, I, EPC, CAP, SI, act=AF.Silu, compile_=True):
    KT = H // 128         # H contraction tiles
    MT = I // 128         # I tiles
    CT = CAP // 128       # token tiles per expert
    N4 = max(H // 512, 1)
    NW = min(H, 512)
    SIT = SI // 128       # shared-intermediate tiles
    TOUT = T // NCORES    # own token slice
    TS = TOUT // 128
    CHUNKS = _chunks(CAP)
    MAXCW = max(CHUNKS)

    nc = bacc.Bacc("TRN2")
    x16 = nc.dram_tensor("x16", [T, H], FP16, kind="ExternalInput")
    xTc = nc.dram_tensor("xTc", [128, KT * TOUT], FP16, kind="ExternalInput")
    w13 = nc.dram_tensor("w13", [EPC, MT, 128, KT * 256], FP16, kind="ExternalInput")
    w2b = nc.dram_tensor("w2b", [EPC, N4, 128, MT * NW], FP16, kind="ExternalInput")
    sw13 = nc.dram_tensor("sw13", [SIT, 128, KT * 256], FP16, kind="ExternalInput")
    sw2b = nc.dram_tensor("sw2b", [N4, 128, SIT * NW], FP16, kind="ExternalInput")
    idx = nc.dram_tensor("idx", [EPC, 128, CAP // 16], I16, kind="ExternalInput")
    idxs = nc.dram_tensor("idxs", [EPC, 128, CAP // 16], I16, kind="ExternalInput")
    gat = nc.dram_tensor("gat", [EPC, 128, CT], F32, kind="ExternalInput")
    out = nc.dram_tensor("out", [TOUT, H], F32, kind="ExternalOutput")

    y_n = [nc.dram_tensor(f"y_col{n}", [T + 128, NW], FP16) for n in range(N4)]
    rs_n = [nc.dram_tensor(f"rs_col{n}", [TOUT, NW], FP16) for n in range(N4)]

    with tile.TileContext(nc) as tc:
        with (
            tc.tile_pool(name="const", bufs=1) as const,
            tc.tile_pool(name="persist", bufs=1) as persist,
            tc.tile_pool(name="xgtp", bufs=2) as xgtp,
            tc.tile_pool(name="xgtp1", bufs=1) as xgtp1,
            tc.tile_pool(name="gp", bufs=2) as gp,
            tc.tile_pool(name="stream", bufs=2) as stream,
            tc.tile_pool(name="one", bufs=1) as one,
            tc.tile_pool(name="small", bufs=2) as small,
            tc.tile_pool(name="psum", bufs=2, space="PSUM") as psum,
        ):
            idx_sb = const.tile([128, EPC, CAP // 16], I16)
            nc.sync.dma_start(idx_sb[:], idx.rearrange("e p c -> p e c"))
            idxs_sb = const.tile([128, EPC, CAP // 16], I16)
            nc.sync.dma_start(idxs_sb[:], idxs.rearrange("e p c -> p e c"))
            gat_sb = const.tile([128, EPC, CT], F32)
            nc.sync.dma_start(gat_sb[:], gat.rearrange("e p c -> p e c"))
            # shared-expert inputs, loaded up-front (Sync queue)
            xtc_sb = persist.tile([128, KT, TOUT], FP16)
            xtc_view = xTc.rearrange("p (k t) -> p k t", t=TOUT)
            for k in range(KT):
                nc.scalar.dma_start(xtc_sb[:, k:k + 1, :], xtc_view[:, k:k + 1, :])
            gs = persist.tile([128, SIT, TOUT], FP16)

            # ---------------- shared intermediate first (hides gather latency)
            for sm in range(SIT // 2):
                s13 = stream.tile([128, KT, 256], FP16, tag="s13")
                nc.scalar.dma_start(
                    s13[:], sw13[sm].rearrange("p (k c) -> p k c", c=256))
                p1 = psum.tile([128, 512], F32, tag="p1")
                p3 = psum.tile([128, 512], F32, tag="p3")
                for k in range(KT):
                    nc.tensor.matmul(p1[:, :TOUT], s13[:, k, :128], xtc_sb[:, k, :],
                                     start=(k == 0), stop=(k == KT - 1))
                for k in range(KT):
                    nc.tensor.matmul(p3[:, :TOUT], s13[:, k, 128:], xtc_sb[:, k, :],
                                     start=(k == 0), stop=(k == KT - 1))
                nc.scalar.activation(gs[:, sm, :], p1[:, :TOUT], act)
                nc.vector.tensor_tensor(gs[:, sm, :], gs[:, sm, :], p3[:, :TOUT],
                                        OP.mult)
            ztile = const.tile([128, NW], FP16)
            nc.vector.memset(ztile[:], 0.0)
            for n in range(N4):
                for b in range(T // 128 + 1):
                    nc.scalar.dma_start(y_n[n][b * 128:(b + 1) * 128, :], ztile[:])
            # ---------------- routed experts --------------------------------
            for e in range(EPC):
                xgt_c = []
                c0 = 0
                for ci, cw in enumerate(CHUNKS):
                    pool_ci = xgtp if ci == 0 else xgtp1
                    xgt = pool_ci.tile([128, KT, cw], FP16, tag=f"xgt{ci}")
                    nc.gpsimd.dma_gather(
                        xgt[:], x16[:],
                        idx_sb[:, e, c0 // 16:(c0 + cw) // 16],
                        cw, cw, H, transpose=True)
                    xgt_c.append(xgt)
                    c0 += cw
                g = gp.tile([128, MT, CAP], FP16, tag="g")
                for m in range(MT):
                    w13t = stream.tile([128, KT, 256], FP16, tag="w13t")
                    nc.sync.dma_start(
                        w13t[:], w13[e, m].rearrange("p (k c) -> p k c", c=256))
                    c0 = 0
                    for ci, cw in enumerate(CHUNKS):
                        p1 = psum.tile([128, 512], F32, tag="p1")
                        p3 = psum.tile([128, 512], F32, tag="p3")
                        for k in range(KT):
                            nc.tensor.matmul(p1[:, :cw], w13t[:, k, :128],
                                             xgt_c[ci][:, k, :cw],
                                             start=(k == 0), stop=(k == KT - 1))
                        for k in range(KT):
                            nc.tensor.matmul(p3[:, :cw], w13t[:, k, 128:],
                                             xgt_c[ci][:, k, :cw],
                                             start=(k == 0), stop=(k == KT - 1))
                        nc.scalar.activation(g[:, m, c0:c0 + cw], p1[:, :cw], act)
                        nc.vector.tensor_tensor(g[:, m, c0:c0 + cw],
                                                g[:, m, c0:c0 + cw],
                                                p3[:, :cw], OP.mult)
                        c0 += cw
                for n in range(N4):
                    w2t = stream.tile([128, MT, NW], FP16, tag="w2t")
                    nc.sync.dma_start(
                        w2t[:], w2b[e, n].rearrange("p (k c) -> p k c", c=NW))
                    yb = stream.tile([128, CT, NW], FP16, tag="yb")
                    for ct in range(CT):
                        p4 = psum.tile([128, NW], F32, tag="p4")
                        for k2 in range(MT):
                            nc.tensor.matmul(p4[:], g[:, k2, ct * 128:(ct + 1) * 128],
                                             w2t[:, k2, :],
                                             start=(k2 == 0), stop=(k2 == MT - 1))
                        nc.vector.tensor_tensor(
                            yb[:, ct, :], p4[:],
                            gat_sb[:, e, ct:ct + 1].to_broadcast([128, NW]),
                            OP.mult)
                    nc.gpsimd.dma_scatter_add(
                        y_n[n][:], yb[:], idxs_sb[:, e, :], CAP, CAP, NW)

            # ---------------- shared intermediate (overlaps nothing yet) ----

            # second half of shared intermediate: fills the RS_0 wait
            for sm in range(SIT // 2, SIT):
                s13 = stream.tile([128, KT, 256], FP16, tag="s13")
                nc.scalar.dma_start(
                    s13[:], sw13[sm].rearrange("p (k c) -> p k c", c=256))
                p1 = psum.tile([128, 512], F32, tag="p1")
                p3 = psum.tile([128, 512], F32, tag="p3")
                for k in range(KT):
                    nc.tensor.matmul(p1[:, :TOUT], s13[:, k, :128], xtc_sb[:, k, :],
                                     start=(k == 0), stop=(k == KT - 1))
                for k in range(KT):
                    nc.tensor.matmul(p3[:, :TOUT], s13[:, k, 128:], xtc_sb[:, k, :],
                                     start=(k == 0), stop=(k == KT - 1))
                nc.scalar.activation(gs[:, sm, :], p1[:, :TOUT], act)
                nc.vector.tensor_tensor(gs[:, sm, :], gs[:, sm, :], p3[:, :TOUT],
                                        OP.mult)

            # ---------------- combine: 4x ReduceScatter (routed only) -------
            for n in range(N4):
                nc.gpsimd.collective_compute(
                    "ReduceScatter", OP.add,
                    replica_groups=[list(range(NCORES))],
                    ins=[y_n[n][0:T, :]],
                    outs=[rs_n[n][:]],
                )

            # ---------------- shared out + combine with rs ------------------
            for n in range(N4):
                s2 = stream.tile([128, SIT, NW], FP16, tag="s2")
                nc.scalar.dma_start(
                    s2[:], sw2b[n].rearrange("p (k c) -> p k c", c=NW))
                for ts in range(TS):
                    po = psum.tile([128, NW], F32, tag="p4")
                    for k2 in range(SIT):
                        nc.tensor.matmul(po[:], gs[:, k2, ts * 128:(ts + 1) * 128],
                                         s2[:, k2, :],
                                         start=(k2 == 0), stop=(k2 == SIT - 1))
                    rst = small.tile([128, NW], FP16, tag="rst")
                    nc.scalar.dma_start(rst[:], rs_n[n][ts * 128:(ts + 1) * 128, :])
                    ott = small.tile([128, NW], F32, tag="ott")
                    nc.vector.tensor_tensor(ott[:], po[:], rst[:], OP.add)
                    nc.sync.dma_start(
                        out[ts * 128:(ts + 1) * 128, n * NW:(n + 1) * NW], ott[:])

    if compile_:
        nc.compile()
    else:
        nc.insert_library_loads()
    return nc


def host_prep(hidden_states, gate_weight, w1, w2, w3, sw1, sw2, sw3):
    B, S, H = hidden_states.shape
    T = B * S
    E, I = w1.shape[0], w1.shape[1]
    SI = sw1.shape[0]
    EPC = E // NCORES
    KT, MT, SIT = H // 128, I // 128, SI // 128
    N4 = max(H // 512, 1)
    NW = min(H, 512)
    TOUT = T // NCORES

    x = np.ascontiguousarray(hidden_states.reshape(T, H), dtype=np.float32)
    combine = _routing(x, gate_weight.astype(np.float32))
    tok_lists = [np.nonzero(combine[:, e])[0] for e in range(E)]
    max_c = max(len(t) for t in tok_lists)
    CAP = max(128, ((max_c + 127) // 128) * 128)
    CT = CAP // 128

    x16 = x.astype(np.float16)
    xT = x.T  # [H, T] view

    s1 = sw1.T.reshape(KT, 128, SIT, 128).transpose(2, 1, 0, 3)
    s3 = sw3.T.reshape(KT, 128, SIT, 128).transpose(2, 1, 0, 3)
    sw13 = np.ascontiguousarray(
        np.concatenate([s1, s3], axis=-1).reshape(SIT, 128, -1), dtype=np.float16)
    sw2b = np.ascontiguousarray(
        sw2.T.reshape(SIT, 128, N4, NW).transpose(2, 1, 0, 3).reshape(N4, 128, -1),
        dtype=np.float16)

    in_maps = []
    for c in range(NCORES):
        els = list(range(c * EPC, (c + 1) * EPC))
        idx_np = np.zeros((EPC, 128, CAP // 16), np.int16)
        idxs_np = np.zeros((EPC, 128, CAP // 16), np.int16)
        gat_np = np.zeros((EPC, 128, CT), np.float32)
        for j, e in enumerate(els):
            toks = tok_lists[e]
            a = np.zeros(CAP, np.int16)
            a[:len(toks)] = toks
            idx_np[j] = np.tile(a.reshape(CAP // 16, 16).T, (8, 1))
            b2 = np.full(CAP, T, np.int16)
            b2[:len(toks)] = toks
            idxs_np[j] = np.tile(b2.reshape(CAP // 16, 16).T, (8, 1))
            gv = np.zeros(CAP, np.float32)
            gv[:len(toks)] = combine[toks, e]
            gat_np[j] = gv.reshape(CT, 128).T
        w13c = np.empty((EPC, MT, 128, KT * 256), np.float16)
        w2c = np.empty((EPC, N4, 128, MT * NW), np.float16)
        for j, e in enumerate(els):
            a1 = w1[e].T.reshape(KT, 128, MT, 128).transpose(2, 1, 0, 3)
            a3 = w3[e].T.reshape(KT, 128, MT, 128).transpose(2, 1, 0, 3)
            w13c[j] = np.concatenate([a1, a3], axis=-1).reshape(MT, 128, -1)
            w2c[j] = (w2[e].T.reshape(MT, 128, N4, NW)
                      .transpose(2, 1, 0, 3).reshape(N4, 128, -1))
        xTc = np.ascontiguousarray(
            xT[:, c * TOUT:(c + 1) * TOUT].reshape(KT, 128, TOUT)
            .transpose(1, 0, 2).reshape(128, -1), dtype=np.float16)
        in_maps.append({
            "x16": x16, "xTc": xTc,
            "w13": w13c, "w2b": w2c,
            "sw13": sw13, "sw2b": sw2b,
            "idx": idx_np, "idxs": idxs_np, "gat": gat_np,
        })
    cfg = dict(T=T, H=H, I=I, EPC=EPC, CAP=CAP, SI=SI)
    return in_maps, cfg


def kernel(**inputs):
    inputs = {k: np.asarray(v) for k, v in inputs.items()}
    hs = inputs["hidden_states"]
    B, S, H = hs.shape
    in_maps, cfg = host_prep(
        hs, inputs["gate_weight"], inputs["w1"], inputs["w2"], inputs["w3"],
        inputs["sw1"], inputs["sw2"], inputs["sw3"])
    nc = build_kernel(**cfg)
    res = run_bass_kernel_spmd(nc, in_maps, list(range(NCORES)))
    y = np.concatenate([res.results[c]["out"] for c in range(NCORES)], axis=0)
    return y.reshape(B, S, H).astype(np.float32)


if __name__ == "__main__":
    pass
